# revision 1
# baseline (speedup 1.0000x reference)
"""Multi-head attention (B=8, N=1024, C=768, H=12) on 8 TRN2 NeuronCores.

Sharding: data-parallel — one batch element per core, weights replicated.
No collectives.

Per-core dataflow (all layouts chosen so no cross-partition moves are needed):
  phase 1: Q.T, K.T [768,1024] and V [1024, 12 heads x (64+1)] via matmuls
           from host-pretransposed x.T and w.T.  V carries a ones-column per
           head so the PV matmul emits the softmax denominator for free.
  phase 2: per head pair (2t at partitions 0:64, 2t+1 at 64:128):
           S.T[j,i] = K.T_tile.T @ Q.T  (two K=64 matmuls row-packed into the
           two PE halves), exp via ScalarE with the 1/sqrt(hd) scale fused
           (no max subtraction: logits are ~N(0,1), exp cannot overflow),
           then O.T[65,1024] += V_aug.T @ P.T streamed over j-tiles.
           Row 64 of O.T is the denominator; reciprocal + partition-broadcast
           + multiply normalizes into OT_sb.
  phase 3: y[i,c_out] = OT.T @ w_proj.T + bias, written straight to DRAM.
"""

import numpy as np

import concourse.bacc as bacc
import concourse.mybir as mybir
import concourse.tile as tile
from concourse.bass_utils import run_bass_kernel_spmd

F32 = mybir.dt.float32
BF16 = mybir.dt.bfloat16
F32R = mybir.dt.float32r

B, N, C = 8, 1024, 768
H, HD = 12, 64
SCALE = HD ** -0.5
NT_I = N // 128   # 8 i/j tiles
NT_C = C // 128   # 6 c tiles
NPAIR = H // 2    # 6 head pairs

# matmul operand dtype: "f32" (safe), "f32r" (4x faster, tf32-ish), "bf16"
MM_MODE = "f32r"


def _mm_dt(mode):
    return {"f32": F32, "f32r": F32R, "bf16": BF16}[mode]


def build(mode=MM_MODE, debug=False):
    dt = _mm_dt(mode)
    np_in_dt = np.float32 if mode in ("f32", "f32r") else np.float32  # dram dtype below
    dram_dt = BF16 if mode == "bf16" else (F32R if mode == "f32r" else F32)

    nc = bacc.Bacc(None, target_bir_lowering=False)

    xt = nc.dram_tensor("xt", [C, N], dram_dt, kind="ExternalInput")
    wq = nc.dram_tensor("wq", [C, C], dram_dt, kind="ExternalInput")
    wk = nc.dram_tensor("wk", [C, C], dram_dt, kind="ExternalInput")
    wv = nc.dram_tensor("wv", [C, C], dram_dt, kind="ExternalInput")
    wp = nc.dram_tensor("wp", [C, C], dram_dt, kind="ExternalInput")
    bias = nc.dram_tensor("bias", [128, C], F32, kind="ExternalInput")
    y = nc.dram_tensor("y", [N, C], F32, kind="ExternalOutput")
    if debug:
        d_qt = nc.dram_tensor("d_qt", [128, NT_C, N], F32, kind="ExternalOutput")
        d_kt = nc.dram_tensor("d_kt", [128, NT_C, N], F32, kind="ExternalOutput")
        d_v = nc.dram_tensor("d_v", [128, NT_I, H, HD + 1], F32, kind="ExternalOutput")
        d_ot = nc.dram_tensor("d_ot", [128, NT_C, N], F32, kind="ExternalOutput")
        d_ops = nc.dram_tensor("d_ops", [4, 65, 512], F32, kind="ExternalOutput")
        d_rl = nc.dram_tensor("d_rl", [2, N], F32, kind="ExternalOutput")
        d_bc = nc.dram_tensor("d_bc", [2, 128, N], F32, kind="ExternalOutput")

    from contextlib import ExitStack
    with tile.TileContext(nc) as tc, ExitStack() as stack:
        pp = stack.enter_context(tc.tile_pool(name="persist", bufs=1))
        with tc.tile_pool(name="psum1", bufs=4, space="PSUM") as ps1:
            qt_sb = pp.tile([128, NT_C, N], dt)      # Q.T rows c -> [c%128, c//128, i]
            kt_sb = pp.tile([128, NT_C, N], dt)
            v_sb = pp.tile([128, NT_I, H, HD + 1], dt)
            ot_sb = pp.tile([128, NT_C, N], dt)      # normalized O.T, same tiling as qt
            wp_sb = pp.tile([128, NT_C, C], dt)
            bias_sb = pp.tile([128, C], F32)

            nc.sync.dma_start(wp_sb[:], wp.rearrange("(t p) d -> p t d", p=128))
            nc.sync.dma_start(bias_sb[:], bias[:])
            ones_view = v_sb[:, :, :, HD:HD + 1]
            if dt == F32R:
                ones_view = ones_view.bitcast(F32)
            nc.vector.memset(ones_view, 1.0)

            with tc.tile_pool(name="xtp", bufs=1) as px:
                xt_sb = px.tile([128, NT_C, N], dt)
                nc.sync.dma_start(xt_sb[:], xt.rearrange("(t p) i -> p t i", p=128))

                # ---- phase 1a: Q.T and K.T ----
                with tc.tile_pool(name="wqk", bufs=1) as p1a:
                    wq_sb = p1a.tile([128, NT_C, C], dt)
                    wk_sb = p1a.tile([128, NT_C, C], dt)
                    nc.sync.dma_start(wq_sb[:], wq.rearrange("(t p) d -> p t d", p=128))
                    nc.sync.dma_start(wk_sb[:], wk.rearrange("(t p) d -> p t d", p=128))

                    for w_sb, out_sb in ((wq_sb, qt_sb), (wk_sb, kt_sb)):
                        for t in range(NT_C):
                            for ch in range(2):
                                acc = ps1.tile([128, 512], F32)
                                for k in range(NT_C):
                                    nc.tensor.matmul(
                                        acc[:],
                                        w_sb[:, k, t * 128:(t + 1) * 128],
                                        xt_sb[:, k, ch * 512:(ch + 1) * 512],
                                        start=(k == 0), stop=(k == NT_C - 1),
                                    )
                                nc.vector.tensor_copy(
                                    out_sb[:, t, ch * 512:(ch + 1) * 512], acc[:]
                                )

                # ---- phase 1b: V (head-strided with ones column) ----
                with tc.tile_pool(name="wvp", bufs=1) as p1b:
                    wv_sb = p1b.tile([128, NT_C, C], dt)
                    nc.sync.dma_start(wv_sb[:], wv.rearrange("(t p) d -> p t d", p=128))
                    for jt in range(NT_I):
                        for ch in range(2):
                            acc = ps1.tile([128, 384], F32)
                            for k in range(NT_C):
                                nc.tensor.matmul(
                                    acc[:],
                                    xt_sb[:, k, jt * 128:(jt + 1) * 128],
                                    wv_sb[:, k, ch * 384:(ch + 1) * 384],
                                    start=(k == 0), stop=(k == NT_C - 1),
                                )
                            nc.vector.tensor_copy(
                                v_sb[:, jt, 6 * ch:6 * ch + 6, 0:HD],
                                acc[:].rearrange("p (h e) -> p h e", e=HD),
                            )

        # ---- phase 2: attention per head pair ----
        with (
            tc.tile_pool(name="att", bufs=2) as p2,
            tc.tile_pool(name="attn1", bufs=1) as p2b,
            tc.tile_pool(name="st", bufs=4, space="PSUM") as ps_st,
            tc.tile_pool(name="ov", bufs=4, space="PSUM") as ps_ov,
        ):
            for t in range(NPAIR):
                hA, hB = 2 * t, 2 * t + 1
                oA = [ps_ov.tile([HD + 1, 512], F32, tag="ov", name=f"oA{t}_{c}")
                      for c in range(2)]
                oB = [ps_ov.tile([HD + 1, 512], F32, tag="ov", name=f"oB{t}_{c}")
                      for c in range(2)]
                for jt in range(NT_I):
                    ptA = p2.tile([128, N], dt, tag="pt")
                    ptB = p2.tile([128, N], dt, tag="pt")
                    for base, pt in ((0, ptA), (64, ptB)):
                        for ch in range(2):
                            st = ps_st.tile([128, 512], F32, tag="st")
                            nc.tensor.matmul(
                                st[:],
                                kt_sb[base:base + 64, t, jt * 128:(jt + 1) * 128],
                                qt_sb[base:base + 64, t, ch * 512:(ch + 1) * 512],
                            )
                            nc.scalar.activation(
                                pt[:, ch * 512:(ch + 1) * 512], st[:],
                                mybir.ActivationFunctionType.Exp, scale=SCALE,
                            )
                    for h, pt, o in ((hA, ptA, oA), (hB, ptB, oB)):
                        for ch in range(2):
                            nc.tensor.matmul(
                                o[ch][:],
                                v_sb[:, jt, h, :],
                                pt[:, ch * 512:(ch + 1) * 512],
                                start=(jt == 0), stop=(jt == NT_I - 1),
                            )
                # denominators -> reciprocal -> broadcast -> normalize
                # (reciprocal shifts l from psum row 64 to sbuf row 0:
                # partition_broadcast on HW always reads partition 0)
                if debug and t == NPAIR - 1:
                    for oi, o1 in enumerate(oA + oB):
                        odump = p2b.tile([65, 512], F32, tag="odump",
                                         name=f"od{oi}")
                        nc.vector.tensor_copy(odump[:], o1[:])
                        nc.sync.dma_start(d_ops[oi], odump[:])
                for di, (base, o) in enumerate(((0, oA), (64, oB))):
                    rl = p2b.tile([128, N], F32, tag="rl")
                    bc = p2b.tile([128, N], F32, tag="bc")
                    for ch in range(2):
                        nc.vector.tensor_copy(
                            rl[0:1, ch * 512:(ch + 1) * 512], o[ch][64:65, :]
                        )
                    nc.vector.reciprocal(bc[0:1, :], rl[0:1, :])
                    # broadcast row 0 -> all 128 partitions by DMA doubling
                    for sh in range(7):
                        w = 1 << sh
                        nc.sync.dma_start(bc[w:2 * w, :], bc[0:w, :])
                    for ch in range(2):
                        nc.vector.tensor_mul(
                            ot_sb[base:base + 64, t, ch * 512:(ch + 1) * 512],
                            o[ch][0:64, :],
                            bc[base:base + 64, ch * 512:(ch + 1) * 512],
                        )

        if debug:
            with tc.tile_pool(name="dbg", bufs=1) as pd:
                for src_sb, dst in ((qt_sb, d_qt), (kt_sb, d_kt), (ot_sb, d_ot)):
                    dt_dump = pd.tile([128, NT_C * N], F32, tag="dump")
                    nc.vector.tensor_copy(
                        dt_dump[:], src_sb[:].rearrange("p a b -> p (a b)"))
                    nc.sync.dma_start(
                        dst.rearrange("p a b -> p (a b)"), dt_dump[:])
                v_dump = pd.tile([128, NT_I * H * (HD + 1)], F32, tag="dump")
                nc.vector.tensor_copy(
                    v_dump[:], v_sb[:].rearrange("p a b c -> p (a b c)"))
                nc.sync.dma_start(
                    d_v.rearrange("p a b c -> p (a b c)"), v_dump[:])

        # ---- phase 3: projection + bias ----
        with (
            tc.tile_pool(name="proj", bufs=2) as p3,
            tc.tile_pool(name="psum3", bufs=4, space="PSUM") as ps3,
        ):
            for it in range(NT_I):
                y_sb = p3.tile([128, C], F32, tag="y")
                for ch in range(2):
                    acc = ps3.tile([128, 384], F32)
                    for k in range(NT_C):
                        nc.tensor.matmul(
                            acc[:],
                            ot_sb[:, k, it * 128:(it + 1) * 128],
                            wp_sb[:, k, ch * 384:(ch + 1) * 384],
                            start=(k == 0), stop=(k == NT_C - 1),
                        )
                    nc.vector.tensor_add(
                        y_sb[:, ch * 384:(ch + 1) * 384], acc[:],
                        bias_sb[:, ch * 384:(ch + 1) * 384],
                    )
                nc.sync.dma_start(y[it * 128:(it + 1) * 128, :], y_sb[:])

    nc.compile()
    nc.finalize()
    return nc


_NC_CACHE = {}


def _get_nc(mode):
    if mode not in _NC_CACHE:
        _NC_CACHE[mode] = build(mode)
    return _NC_CACHE[mode]


def _prep_host(x, w_qkv, w_proj, b_proj, mode):
    np_dt = np.float32
    cast = (lambda a: a.astype(ml_bf16)) if mode == "bf16" else (lambda a: a.astype(np_dt))
    if mode == "bf16":
        import ml_dtypes
        global ml_bf16
        ml_bf16 = ml_dtypes.bfloat16
    xt = np.ascontiguousarray(x.transpose(0, 2, 1))          # [B, C, N]
    wq_t = np.ascontiguousarray(w_qkv[0:C].T)                # [C, C] c_in-major
    wk_t = np.ascontiguousarray(w_qkv[C:2 * C].T)
    wv_t = np.ascontiguousarray(w_qkv[2 * C:3 * C].T)
    wp_t = np.ascontiguousarray(w_proj.T)
    bias_rep = np.ascontiguousarray(
        np.broadcast_to(b_proj.astype(np.float32), (128, C))
    )
    return (cast(xt), cast(wq_t), cast(wk_t), cast(wv_t), cast(wp_t), bias_rep)


def run(x, w_qkv, w_proj, b_proj, mode=MM_MODE, trace=False):
    nc = _get_nc(mode)
    xt, wq_t, wk_t, wv_t, wp_t, bias_rep = _prep_host(x, w_qkv, w_proj, b_proj, mode)
    in_maps = [
        {"xt": np.ascontiguousarray(xt[b]), "wq": wq_t, "wk": wk_t,
         "wv": wv_t, "wp": wp_t, "bias": bias_rep}
        for b in range(B)
    ]
    res = run_bass_kernel_spmd(
        nc, in_maps, core_ids=list(range(B)), trace=trace
    )
    out = np.stack([res.results[b]["y"] for b in range(B)]).astype(np.float32)
    return out, res


def kernel(x, w_qkv, w_proj, b_proj):
    out, _ = run(x, w_qkv, w_proj, b_proj)
    return out



# revision 4
# speedup vs baseline: 3.0276x; 3.0276x over previous
"""Multi-head attention (B=8, N=1024, C=768, H=12) on 8 TRN2 NeuronCores.

Sharding: data-parallel - one batch element per core, weights replicated.
No collectives.

v2 design (vs baseline): bf16 matmul operands everywhere (f32 PSUM), query
dim split in 512-halves so PSUM fits 8 banks with QKV interleave slots,
software-pipelined S->exp->PV per j-tile, QKV/V generation interleaved into
the attention loop as PE gap-filler (keeps HAM warm), reciprocal via the
fast custom-DVE approx, partition broadcast on GPSIMD instead of 7 chained
DMAs.

Per-core dataflow:
  qt/kt [128, pair, 1024]: rows = head-pair dims (A at 0:64, B at 64:128).
  v_sb [128 j, jt, head, 65]: col 64 is ones -> PV row 64 = softmax denom.
  Per (pair t, half ib, jtile): S^T halves via two K=64 matmuls row-packed
  into the PE halves, one exp ACTIVATE (FD=1024, scale fused), PV accumulates
  O^T[65, 512] over jt. Normalize: denom row -> reciprocal_approx_fast ->
  partition_broadcast -> multiply into ot_sb (bf16).
  Proj: y = OT.T @ wp + bias per 128-row tile, straight to DRAM.
"""

from contextlib import ExitStack

import numpy as np

import concourse.bacc as bacc
import concourse.mybir as mybir
import concourse.tile as tile
from concourse.bass_utils import run_bass_kernel_spmd

F32 = mybir.dt.float32
BF16 = mybir.dt.bfloat16

B, N, C = 8, 1024, 768
H, HD = 12, 64
SCALE = HD ** -0.5
NT_I = N // 128   # 8 i/j tiles
NT_C = C // 128   # 6 c tiles (== head pairs)
NPAIR = H // 2    # 6


def build():
    nc = bacc.Bacc(None, target_bir_lowering=False)

    xt = nc.dram_tensor("xt", [C, N], BF16, kind="ExternalInput")
    wq = nc.dram_tensor("wq", [C, C], BF16, kind="ExternalInput")
    wk = nc.dram_tensor("wk", [C, C], BF16, kind="ExternalInput")
    wv = nc.dram_tensor("wv", [C, C], BF16, kind="ExternalInput")
    wp = nc.dram_tensor("wp", [C, C], BF16, kind="ExternalInput")
    bias = nc.dram_tensor("bias", [128, C], F32, kind="ExternalInput")
    y = nc.dram_tensor("y", [N, C], F32, kind="ExternalOutput")

    with tile.TileContext(nc) as tc, ExitStack() as stack:
        pp = stack.enter_context(tc.tile_pool(name="persist", bufs=1))
        p_pt = stack.enter_context(tc.tile_pool(name="pt", bufs=4))
        p_nrm = stack.enter_context(tc.tile_pool(name="nrm", bufs=2))
        p_y = stack.enter_context(tc.tile_pool(name="yout", bufs=2))
        ps_qkv = stack.enter_context(
            tc.tile_pool(name="psq", bufs=2, space="PSUM"))
        ps_st = stack.enter_context(
            tc.tile_pool(name="psst", bufs=2, space="PSUM"))
        ps_ov = stack.enter_context(
            tc.tile_pool(name="psov", bufs=2, space="PSUM"))

        xt_sb = pp.tile([128, NT_C, N], BF16)
        wq_sb = pp.tile([128, NT_C, C], BF16)
        wk_sb = pp.tile([128, NT_C, C], BF16)
        wv_sb = pp.tile([128, NT_C, C], BF16)
        wp_sb = pp.tile([128, NT_C, C], BF16)
        bias_sb = pp.tile([128, C], F32)
        qt_sb = pp.tile([128, NPAIR, N], BF16)
        kt_sb = pp.tile([128, NPAIR, N], BF16)
        v_sb = pp.tile([128, NT_I, H, HD + 1], BF16)
        ot_sb = pp.tile([128, NPAIR, N], BF16)

        nc.sync.dma_start(xt_sb[:], xt.rearrange("(t p) i -> p t i", p=128))
        nc.sync.dma_start(wq_sb[:], wq.rearrange("(t p) d -> p t d", p=128))
        nc.sync.dma_start(wk_sb[:], wk.rearrange("(t p) d -> p t d", p=128))
        nc.sync.dma_start(wv_sb[:], wv.rearrange("(t p) d -> p t d", p=128))
        nc.sync.dma_start(wp_sb[:], wp.rearrange("(t p) d -> p t d", p=128))
        nc.sync.dma_start(bias_sb[:], bias[:])
        nc.vector.memset(v_sb[:, :, :, HD:HD + 1], 1.0)

        def gen_qk_chunk(t, which, ch):
            """One accumulation chain of Q.T (which=0) or K.T (which=1)."""
            w_sb, out_sb = ((wq_sb, qt_sb), (wk_sb, kt_sb))[which]
            acc = ps_qkv.tile([128, 512], F32, tag="acc",
                              name=f"qk{t}_{which}_{ch}")
            for k in range(NT_C):
                nc.tensor.matmul(
                    acc[:],
                    w_sb[:, k, t * 128:(t + 1) * 128],
                    xt_sb[:, k, ch * 512:(ch + 1) * 512],
                    start=(k == 0), stop=(k == NT_C - 1),
                )
            nc.vector.tensor_copy(out_sb[:, t, ch * 512:(ch + 1) * 512],
                                  acc[:])

        def gen_v_chunk(jt, ch):
            acc = ps_qkv.tile([128, 384], F32, tag="acc",
                              name=f"v{jt}_{ch}")
            for k in range(NT_C):
                nc.tensor.matmul(
                    acc[:],
                    xt_sb[:, k, jt * 128:(jt + 1) * 128],
                    wv_sb[:, k, ch * 384:(ch + 1) * 384],
                    start=(k == 0), stop=(k == NT_C - 1),
                )
            nc.vector.tensor_copy(
                v_sb[:, jt, 6 * ch:6 * ch + 6, 0:HD],
                acc[:].rearrange("p (h e) -> p h e", e=HD),
            )

        def attn_pair(t, ib, filler):
            """Attention for head pair t on query half ib (512 queries).

            filler: list of zero-arg emitters (extra PE work) drained a few
            per jt step so the scheduler has gap-fill matmuls while ACT
            runs exp. Drained fast enough that all run by step NT_I-2.
            """
            i0 = ib * 512
            hA, hB = 2 * t, 2 * t + 1
            per_step = -(-len(filler) // (NT_I - 1)) if filler else 0
            ovA = ps_ov.tile([HD + 1, 512], F32, tag="ov",
                             name=f"ovA{t}_{ib}")
            ovB = ps_ov.tile([HD + 1, 512], F32, tag="ov",
                             name=f"ovB{t}_{ib}")
            pts = [None] * NT_I
            for jt in range(NT_I + 1):
                for _ in range(per_step):
                    if filler:
                        filler.pop(0)()
                if jt < NT_I:
                    st = ps_st.tile([128, 1024], F32, tag="st",
                                    name=f"st{t}_{ib}_{jt}")
                    nc.tensor.matmul(
                        st[:, 0:512],
                        kt_sb[0:64, t, jt * 128:(jt + 1) * 128],
                        qt_sb[0:64, t, i0:i0 + 512],
                    )
                    nc.tensor.matmul(
                        st[:, 512:1024],
                        kt_sb[64:128, t, jt * 128:(jt + 1) * 128],
                        qt_sb[64:128, t, i0:i0 + 512],
                    )
                    pt = p_pt.tile([128, 1024], BF16, tag="pt")
                    nc.scalar.activation(
                        pt[:], st[:],
                        mybir.ActivationFunctionType.Exp, scale=SCALE,
                    )
                    pts[jt] = pt
                if jt > 0:
                    j = jt - 1
                    pt = pts[j]
                    nc.tensor.matmul(
                        ovA[:], v_sb[:, j, hA, :], pt[:, 0:512],
                        start=(j == 0), stop=(j == NT_I - 1),
                    )
                    nc.tensor.matmul(
                        ovB[:], v_sb[:, j, hB, :], pt[:, 512:1024],
                        start=(j == 0), stop=(j == NT_I - 1),
                    )
            # normalize: denom row 64 -> 1/l -> broadcast -> multiply
            for base, ov in ((0, ovA), (64, ovB)):
                rl = p_nrm.tile([1, 512], F32, tag="rl")
                rc = p_nrm.tile([1, 512], F32, tag="rc")
                bc = p_nrm.tile([128, 512], F32, tag="bc")
                nc.vector.tensor_copy(rl[0:1, :], ov[64:65, :])
                nc.vector.reciprocal_approx_fast(rc[0:1, :], rl[0:1, :])
                nc.gpsimd.partition_broadcast(bc[:], rc[0:1, :])
                nc.vector.tensor_mul(
                    ot_sb[base:base + 64, t, i0:i0 + 512],
                    ov[0:64, :],
                    bc[base:base + 64, :],
                )

        def proj(it):
            y_sb = p_y.tile([128, C], F32, tag="y")
            for ch in range(2):
                acc = ps_qkv.tile([128, 384], F32, tag="acc",
                                  name=f"p{it}_{ch}")
                for k in range(NT_C):
                    nc.tensor.matmul(
                        acc[:],
                        ot_sb[:, k, it * 128:(it + 1) * 128],
                        wp_sb[:, k, ch * 384:(ch + 1) * 384],
                        start=(k == 0), stop=(k == NT_C - 1),
                    )
                nc.vector.tensor_add(
                    y_sb[:, ch * 384:(ch + 1) * 384], acc[:],
                    bias_sb[:, ch * 384:(ch + 1) * 384],
                )
            nc.sync.dma_start(y[it * 128:(it + 1) * 128, :], y_sb[:])

        # prologue: Q.T/K.T for pair 0 and the first V tiles
        for ch in range(2):
            gen_qk_chunk(0, 0, ch)
            gen_qk_chunk(0, 1, ch)
        gen_v_chunk(0, 0)
        gen_v_chunk(0, 1)

        for t in range(NPAIR):
            if t == 0:
                # remaining V tiles ride inside pair 0's first half; V[j]
                # for both ch lands before the PV that reads it
                fill0 = [lambda jt=jt, ch=ch: gen_v_chunk(jt, ch)
                         for jt in range(1, NT_I) for ch in range(2)]
                attn_pair(t, 0, fill0)
            else:
                attn_pair(t, 0, [])
            # next pair's Q.T/K.T rides inside this pair's second half
            if t + 1 < NPAIR:
                fill1 = [lambda w=w, ch=ch: gen_qk_chunk(t + 1, w, ch)
                         for w in range(2) for ch in range(2)]
            else:
                fill1 = []
            attn_pair(t, 1, fill1)

        for it in range(NT_I):
            proj(it)

    nc.compile()
    nc.finalize()
    return nc


_NC_CACHE = {}


def _get_nc(mode=None):
    if "nc" not in _NC_CACHE:
        _NC_CACHE["nc"] = build()
    return _NC_CACHE["nc"]


def _prep_host(x, w_qkv, w_proj, b_proj, mode=None):
    import ml_dtypes
    bf16 = ml_dtypes.bfloat16

    xt = np.ascontiguousarray(
        np.asarray(x).transpose(0, 2, 1)).astype(bf16)       # [B, C, N]
    wq_t = np.ascontiguousarray(w_qkv[0:C].T).astype(bf16)   # [C, C]
    wk_t = np.ascontiguousarray(w_qkv[C:2 * C].T).astype(bf16)
    wv_t = np.ascontiguousarray(w_qkv[2 * C:3 * C].T).astype(bf16)
    wp_t = np.ascontiguousarray(w_proj.T).astype(bf16)
    bias_rep = np.ascontiguousarray(
        np.broadcast_to(np.asarray(b_proj, dtype=np.float32), (128, C)))
    return xt, wq_t, wk_t, wv_t, wp_t, bias_rep


def run(x, w_qkv, w_proj, b_proj, mode=None, trace=False):
    nc = _get_nc()
    xt, wq_t, wk_t, wv_t, wp_t, bias_rep = _prep_host(x, w_qkv, w_proj, b_proj)
    in_maps = [
        {"xt": np.ascontiguousarray(xt[b]), "wq": wq_t, "wk": wk_t,
         "wv": wv_t, "wp": wp_t, "bias": bias_rep}
        for b in range(B)
    ]
    res = run_bass_kernel_spmd(
        nc, in_maps, core_ids=list(range(B)), trace=trace
    )
    out = np.stack([res.results[b]["y"] for b in range(B)]).astype(np.float32)
    return out, res


def kernel(x, w_qkv, w_proj, b_proj):
    out, _ = run(x, w_qkv, w_proj, b_proj)
    return out
